# revision 8
# baseline (speedup 1.0000x reference)
"""Trainium2 Bass kernel for nn_DetectUDPModel (rank-2 Hermitian detection loss).

Math: the reference computes
    loss = sum_m |v_m|^2,   v_m = lam0 * u0^T B_m conj(u0) - lam1 * u1^T B_m conj(u1)
with B_m = R_m - i*I_m (basis_re/basis_im) and u_j the prepared eigenvectors.
Writing u = ur + i*ui and defining the two real row-vectors over b
    g1 = ur^T R + ui^T I
    g2 = ui^T R - ur^T I
one checks  u^T (R - iI) conj(u) = (g1 + i*g2) . conj(u)  exactly.  So per m
and per eigenvector only TWO device-side output rows are needed, and the R/I
pair is contracted JOINTLY -- a doubled contraction dim (256) that maps onto
the TensorEngine's fp8 DoubleRow perf mode (2 k-tile planes per matmul).

Device pipeline (memory-bound; the 16 per-core DMA engines saturate at
~360 GB/s aggregate, so wire time ~24us is the floor for the 8.4 MiB fp8
stream):
  - basis cast to fp8-e4m3 on host, packed so every chunk DMA is contiguous
    in HBM and lands [128, n_slots, 2, 512] in SBUF (R-plane | I-plane per
    4-m slot).
  - chunk sizes ramp 1-2-4-...-4-2-1 slots so the first matmul starts as
    early as possible and the tail drains fast; chunks round-robin over the
    three DMA rings (sync/scalar HWDGE, gpsimd SWDGE) with ring byte-shares
    tuned for simultaneous finish (gpsimd starts ~3us late).
  - stationary: zero-shifted scaled (ur,ui) pair-column planes, 8 slots per
    32-partition PSUM group; 8 DoubleRow matmuls accumulate one (32, 512)
    PSUM tile = 32 m's.  PSUM -> fp16 SBUF cast on DVE, outputs DMA'd on the
    sync/scalar rings.
  - host stage 2 contracts with exact f64 conj(u) (right-side vectors carry
    no quantization error).

m is sharded across the 8 NeuronCores; per-core partial losses are summed on
host (equivalent to the scalar all-reduce).
"""

import numpy as np
import ml_dtypes

M_TOTAL = 2048
N = 128
N_CORES = 8
M_LOCAL = M_TOTAL // N_CORES       # 256 m's per core
PAIRS_PER_MM = 4                   # m's per matmul slot (out free 512)
MM_PER_GROUP = 8                   # slots accumulated per PSUM tile
GROUP_MS = PAIRS_PER_MM * MM_PER_GROUP   # 32 m's per PSUM group
N_GROUPS = M_LOCAL // GROUP_MS     # 8
N_SLOTS = M_LOCAL // PAIRS_PER_MM  # 64 matmul slots per core
U_SCALE = 32.0                     # keeps u entries in fp8 normal range

# (ring, n_slots) per input chunk, in global slot order.  Rings: 0=sync
# (HWDGE), 1=scalar (HWDGE, also carries the u-load first), 2=gpsimd
# (SWDGE, starts ~3us later -> slightly lighter).  1 slot = 128 KiB fp8.
CHUNK_SCHED = [
    (0, 1), (1, 1), (2, 1),
    (0, 2), (1, 2), (2, 2),
    (0, 4), (1, 4), (2, 4),
    (0, 4), (1, 4), (2, 4),
    (0, 4), (1, 4), (2, 4),
    (0, 4), (1, 4), (2, 4),
    (0, 2), (1, 2), (2, 2),
    (0, 1),
]
assert sum(n for _, n in CHUNK_SCHED) == N_SLOTS
# Per-group output ring.  Output dma_starts are issued AFTER every input
# dma_start (a dma_start whose wait-semaphore is a compute result blocks the
# engine's in-order queue, starving later input chunks -- measured as an
# 11.5us issue stall when outputs were interleaved).  The final group's
# output rides the low-latency sync ring.
OUT_RING_OF_GROUP = (2, 1, 2, 1, 2, 1, 2, 0)
BT_BUFS = 4                        # input tile buffering depth per ring pool
PSUM_BUFS = 4

_CACHE: dict = {}


def _chunk_layout():
    """Per chunk: (ring, n_slots, start_slot, dram_name, index_in_dram)."""
    out = []
    start = 0
    counts: dict = {}
    for ring, n in CHUNK_SCHED:
        idx = counts.get(n, 0)
        counts[n] = idx + 1
        out.append((ring, n, start, f"xs{n}", idx))
        start += n
    return out, counts


def _build_nc():
    """Build + compile the per-core SPMD program."""
    import concourse.bacc as bacc
    import concourse.mybir as mybir
    from concourse import tile

    f8 = mybir.dt.float8e4
    f16 = mybir.dt.float16
    f32 = mybir.dt.float32

    chunks, size_counts = _chunk_layout()

    nc = bacc.Bacc("TRN2", target_bir_lowering=False, debug=False,
                   num_devices=N_CORES)
    xs_in = {
        f"xs{n}": nc.dram_tensor(
            f"xs{n}", [cnt, N, n, 2, PAIRS_PER_MM * N], f8,
            kind="ExternalInput")
        for n, cnt in sorted(size_counts.items())
    }
    # u[a, r, i*32 + col]: zero-shifted stationary plane pair for slot i
    u_in = nc.dram_tensor(
        "u", [N, 2, MM_PER_GROUP * GROUP_MS], f8, kind="ExternalInput")
    t_out = nc.dram_tensor(
        "t_out", [N_GROUPS, GROUP_MS, PAIRS_PER_MM * N], f16,
        kind="ExternalOutput")

    with tile.TileContext(nc) as tc:
        with (
            tc.tile_pool(name="bt0", bufs=BT_BUFS) as bp0,
            tc.tile_pool(name="bt1", bufs=BT_BUFS) as bp1,
            tc.tile_pool(name="bt2", bufs=BT_BUFS) as bp2,
            tc.tile_pool(name="ps", bufs=PSUM_BUFS, space="PSUM") as ppool,
            tc.tile_pool(name="st", bufs=N_GROUPS) as spool,
            tc.tile_pool(name="cn", bufs=1) as cpool,
        ):
            engines = [nc.sync, nc.scalar, nc.gpsimd]
            bpools = [bp0, bp1, bp2]
            u_t = cpool.tile([N, 2, MM_PER_GROUP * GROUP_MS], f8)
            nc.scalar.dma_start(u_t[:], u_in[:])  # scalar ring head
            psum = None
            stages = []
            for ring, n, start, name, idx in chunks:
                bt = bpools[ring].tile([N, n, 2, PAIRS_PER_MM * N], f8)
                engines[ring].dma_start(bt[:], xs_in[name][idx])
                for q in range(n):
                    s = start + q
                    g, i = divmod(s, MM_PER_GROUP)
                    if i == 0:
                        psum = ppool.tile([GROUP_MS, PAIRS_PER_MM * N], f32)
                    nc.tensor.matmul(
                        psum[:],
                        u_t[:, :, i * GROUP_MS:(i + 1) * GROUP_MS],
                        bt[:, q, :, :],
                        start=(i == 0),
                        stop=(i == MM_PER_GROUP - 1),
                        perf_mode=mybir.MatmulPerfMode.DoubleRow,
                    )
                    if i == MM_PER_GROUP - 1:
                        stage = spool.tile(
                            [GROUP_MS, PAIRS_PER_MM * N], f16)
                        nc.vector.tensor_copy(stage[:], psum[:])
                        stages.append(stage)
            for g, stage in enumerate(stages):
                engines[OUT_RING_OF_GROUP[g]].dma_start(t_out[g], stage[:])
    nc.compile()
    return nc


def _get_nc():
    if "nc" not in _CACHE:
        _CACHE["nc"] = _build_nc()
    return _CACHE["nc"]


def _host_prep(theta: np.ndarray, evl: np.ndarray):
    """Eigenvector/eigenvalue prep (tiny, f64 on host)."""
    theta = np.asarray(theta, dtype=np.float64)
    evl = np.asarray(evl, dtype=np.float64)
    c0 = theta[0] + 1j * theta[1]
    evc0 = c0 / np.linalg.norm(c0)
    c1 = theta[2] + 1j * theta[3]
    c1 = c1 - np.vdot(evc0, c1) * evc0
    evc1 = c1 / np.linalg.norm(c1)
    lam = np.log1p(np.exp(evl))
    lam = lam / np.linalg.norm(lam)
    U = np.stack([evc0.real, evc0.imag, evc1.real, evc1.imag], axis=1)
    return U, lam  # f64 (128, 4), f64 (2,)


def _make_u_planes(U: np.ndarray) -> np.ndarray:
    """Zero-shifted DoubleRow stationary planes, fp8, scaled by U_SCALE.

    Slot i covers out partitions 4i..4i+3 = rows [g1_0, g2_0, g1_1, g2_1]:
      plane 0 (applied to R): [ur0, ui0, ur1, ui1]
      plane 1 (applied to I): [ui0, -ur0, ui1, -ur1]
    """
    A = U * U_SCALE
    Bp = np.stack([U[:, 1], -U[:, 0], U[:, 3], -U[:, 2]], axis=1) * U_SCALE
    u_np = np.zeros((N, 2, MM_PER_GROUP, GROUP_MS), dtype=np.float32)
    for i in range(MM_PER_GROUP):
        u_np[:, 0, i, 4 * i:4 * i + 4] = A
        u_np[:, 1, i, 4 * i:4 * i + 4] = Bp
    return np.ascontiguousarray(
        u_np.reshape(N, 2, MM_PER_GROUP * GROUP_MS)
    ).astype(ml_dtypes.float8_e4m3)


def _pack_stream(basis_re_k: np.ndarray, basis_im_k: np.ndarray) -> dict:
    """fp8-cast + pack one core's slice into per-chunk-size xs arrays.

    Slot s covers m = 4s+j;  xs{n}[idx, a, q, r, j*N + b], r=0: R, r=1: I.
    """
    R = np.asarray(basis_re_k, dtype=np.float32).astype(ml_dtypes.float8_e4m3)
    I = np.asarray(basis_im_k, dtype=np.float32).astype(ml_dtypes.float8_e4m3)
    # [s, r, j, a, b]
    X = np.stack([R.reshape(N_SLOTS, PAIRS_PER_MM, N, N),
                  I.reshape(N_SLOTS, PAIRS_PER_MM, N, N)], axis=1)
    chunks, size_counts = _chunk_layout()
    bufs = {n: np.empty((cnt, N, n, 2, PAIRS_PER_MM * N),
                        dtype=ml_dtypes.float8_e4m3)
            for n, cnt in size_counts.items()}
    for ring, n, start, name, idx in chunks:
        blk = X[start:start + n]                    # [q, r, j, a, b]
        blk = np.transpose(blk, (3, 0, 1, 2, 4))    # [a, q, r, j, b]
        bufs[n][idx] = blk.reshape(N, n, 2, PAIRS_PER_MM * N)
    return {f"xs{n}": v for n, v in bufs.items()}


def _decode(t_raw: np.ndarray, U: np.ndarray, lam: np.ndarray) -> float:
    """Host stage 2 + combine for one core's t_out. Returns partial loss."""
    # t_raw[g, 4i + x', j*128 + b], m = g*32 + i*4 + j
    G = t_raw.reshape(N_GROUPS, MM_PER_GROUP, 4, PAIRS_PER_MM, N).astype(
        np.float64)
    G = np.transpose(G, (0, 1, 3, 2, 4)).reshape(M_LOCAL, 4, N) / U_SCALE
    u0 = U[:, 0] + 1j * U[:, 1]
    u1 = U[:, 2] + 1j * U[:, 3]
    F0 = (G[:, 0, :] + 1j * G[:, 1, :]) @ np.conj(u0)
    F1 = (G[:, 2, :] + 1j * G[:, 3, :]) @ np.conj(u1)
    v = lam[0] * F0 - lam[1] * F1
    return float(np.sum(v.real ** 2 + v.imag ** 2))


def _make_in_maps(basis_re, basis_im, theta, evl):
    U, lam = _host_prep(theta, evl)
    u_packed = _make_u_planes(U)
    in_maps = []
    for k in range(N_CORES):
        sl = slice(k * M_LOCAL, (k + 1) * M_LOCAL)
        im = _pack_stream(basis_re[sl], basis_im[sl])
        im["u"] = u_packed
        in_maps.append(im)
    return in_maps, U, lam


def _run_device(in_maps, **kwargs):
    from concourse.bass_utils import run_bass_kernel_spmd
    nc = _get_nc()
    return run_bass_kernel_spmd(nc, in_maps, list(range(N_CORES)), **kwargs)


def kernel(basis_re, basis_im, theta, evl) -> np.ndarray:
    in_maps, U, lam = _make_in_maps(basis_re, basis_im, theta, evl)
    res = _run_device(in_maps)
    total = 0.0
    for k in range(N_CORES):
        total += _decode(res.results[k]["t_out"], U, lam)
    return np.float32(total)


# revision 9
# speedup vs baseline: 1.0415x; 1.0415x over previous
"""Trainium2 Bass kernel for nn_DetectUDPModel (rank-2 Hermitian detection loss).

Math: the reference computes
    loss = sum_m |v_m|^2,   v_m = lam0 * u0^T B_m conj(u0) - lam1 * u1^T B_m conj(u1)
with B_m = R_m - i*I_m (basis_re/basis_im) and u_j the prepared eigenvectors.
Writing u = ur + i*ui and defining the two real row-vectors over b
    g1 = ur^T R + ui^T I
    g2 = ui^T R - ur^T I
one checks  u^T (R - iI) conj(u) = (g1 + i*g2) . conj(u)  exactly.  So per m
and per eigenvector only TWO device-side output rows are needed, and the R/I
pair is contracted JOINTLY -- a doubled contraction dim (256) that maps onto
the TensorEngine's fp8 DoubleRow perf mode (2 k-tile planes per matmul).

Device pipeline (memory-bound; the 16 per-core DMA engines saturate at
~360 GB/s aggregate, so wire time ~24us is the floor for the 8.4 MiB fp8
stream):
  - basis cast to fp8-e4m3 on host, packed so every chunk DMA is contiguous
    in HBM and lands [128, n_slots, 2, 512] in SBUF (R-plane | I-plane per
    4-m slot).
  - chunk sizes ramp 1-2-4-...-4-2-1 slots so the first matmul starts as
    early as possible and the tail drains fast; chunks round-robin over the
    three DMA rings (sync/scalar HWDGE, gpsimd SWDGE) with ring byte-shares
    tuned for simultaneous finish (gpsimd starts ~3us late).
  - stationary: zero-shifted scaled (ur,ui) pair-column planes, 8 slots per
    32-partition PSUM group; 8 DoubleRow matmuls accumulate one (32, 512)
    PSUM tile = 32 m's.  PSUM -> fp16 SBUF cast on DVE, outputs DMA'd on the
    sync/scalar rings.
  - host stage 2 contracts with exact f64 conj(u) (right-side vectors carry
    no quantization error).

m is sharded across the 8 NeuronCores; per-core partial losses are summed on
host (equivalent to the scalar all-reduce).
"""

import numpy as np
import ml_dtypes

M_TOTAL = 2048
N = 128
N_CORES = 8
M_LOCAL = M_TOTAL // N_CORES       # 256 m's per core
PAIRS_PER_MM = 4                   # m's per matmul slot (out free 512)
MM_PER_GROUP = 8                   # slots accumulated per PSUM tile
GROUP_MS = PAIRS_PER_MM * MM_PER_GROUP   # 32 m's per PSUM group
N_GROUPS = M_LOCAL // GROUP_MS     # 8
N_SLOTS = M_LOCAL // PAIRS_PER_MM  # 64 matmul slots per core
U_SCALE = 32.0                     # keeps u entries in fp8 normal range

# (ring, n_slots) per input chunk, in global slot order.  Rings: 0=sync
# (HWDGE), 1=scalar (HWDGE, also carries the u-load first), 2=gpsimd
# (SWDGE, starts ~3us later -> slightly lighter).  1 slot = 128 KiB fp8.
CHUNK_SCHED = [
    (0, 1), (1, 1), (2, 1),
    (0, 4), (1, 4), (2, 4),
    (0, 4), (1, 4), (2, 4),
    (0, 4), (1, 4), (2, 4),
    (0, 4), (1, 4), (2, 4),
    (0, 4), (1, 4), (2, 4),
    (0, 1),
]
assert sum(n for _, n in CHUNK_SCHED) == N_SLOTS
# Per-group output ring.  Output dma_starts are issued AFTER every input
# dma_start (a dma_start whose wait-semaphore is a compute result blocks the
# engine's in-order queue, starving later input chunks -- measured as an
# 11.5us issue stall when outputs were interleaved).  The final group's
# output rides the low-latency sync ring.
OUT_RING_OF_GROUP = (2, 1, 2, 1, 2, 1, 2, 0)
BT_BUFS = 4                        # input tile buffering depth per ring pool
PSUM_BUFS = 4

_CACHE: dict = {}


def _chunk_layout():
    """Per chunk: (ring, n_slots, start_slot, dram_name, index_in_dram)."""
    out = []
    start = 0
    counts: dict = {}
    for ring, n in CHUNK_SCHED:
        idx = counts.get(n, 0)
        counts[n] = idx + 1
        out.append((ring, n, start, f"xs{n}", idx))
        start += n
    return out, counts


def _build_nc():
    """Build + compile the per-core SPMD program."""
    import concourse.bacc as bacc
    import concourse.mybir as mybir
    from concourse import tile

    f8 = mybir.dt.float8e4
    f16 = mybir.dt.float16
    f32 = mybir.dt.float32

    chunks, size_counts = _chunk_layout()

    nc = bacc.Bacc("TRN2", target_bir_lowering=False, debug=False,
                   num_devices=N_CORES)
    xs_in = {
        f"xs{n}": nc.dram_tensor(
            f"xs{n}", [cnt, N, n, 2, PAIRS_PER_MM * N], f8,
            kind="ExternalInput")
        for n, cnt in sorted(size_counts.items())
    }
    # u[a, r, i*32 + col]: zero-shifted stationary plane pair for slot i
    u_in = nc.dram_tensor(
        "u", [N, 2, MM_PER_GROUP * GROUP_MS], f8, kind="ExternalInput")
    t_out = nc.dram_tensor(
        "t_out", [N_GROUPS, GROUP_MS, PAIRS_PER_MM * N], f16,
        kind="ExternalOutput")

    with tile.TileContext(nc) as tc:
        with (
            tc.tile_pool(name="bt0", bufs=BT_BUFS) as bp0,
            tc.tile_pool(name="bt1", bufs=BT_BUFS) as bp1,
            tc.tile_pool(name="bt2", bufs=BT_BUFS) as bp2,
            tc.tile_pool(name="ps", bufs=PSUM_BUFS, space="PSUM") as ppool,
            tc.tile_pool(name="st", bufs=N_GROUPS) as spool,
            tc.tile_pool(name="cn", bufs=1) as cpool,
        ):
            engines = [nc.sync, nc.scalar, nc.gpsimd]
            bpools = [bp0, bp1, bp2]
            u_t = cpool.tile([N, 2, MM_PER_GROUP * GROUP_MS], f8)
            nc.scalar.dma_start(u_t[:], u_in[:])  # scalar ring head
            psum = None
            stages = []
            for ring, n, start, name, idx in chunks:
                bt = bpools[ring].tile([N, n, 2, PAIRS_PER_MM * N], f8)
                engines[ring].dma_start(bt[:], xs_in[name][idx])
                for q in range(n):
                    s = start + q
                    g, i = divmod(s, MM_PER_GROUP)
                    if i == 0:
                        psum = ppool.tile([GROUP_MS, PAIRS_PER_MM * N], f32)
                    nc.tensor.matmul(
                        psum[:],
                        u_t[:, :, i * GROUP_MS:(i + 1) * GROUP_MS],
                        bt[:, q, :, :],
                        start=(i == 0),
                        stop=(i == MM_PER_GROUP - 1),
                        perf_mode=mybir.MatmulPerfMode.DoubleRow,
                    )
                    if i == MM_PER_GROUP - 1:
                        stage = spool.tile(
                            [GROUP_MS, PAIRS_PER_MM * N], f16)
                        nc.vector.tensor_copy(stage[:], psum[:])
                        stages.append(stage)
            for g, stage in enumerate(stages):
                engines[OUT_RING_OF_GROUP[g]].dma_start(t_out[g], stage[:])
    nc.compile()
    return nc


def _get_nc():
    if "nc" not in _CACHE:
        _CACHE["nc"] = _build_nc()
    return _CACHE["nc"]


def _host_prep(theta: np.ndarray, evl: np.ndarray):
    """Eigenvector/eigenvalue prep (tiny, f64 on host)."""
    theta = np.asarray(theta, dtype=np.float64)
    evl = np.asarray(evl, dtype=np.float64)
    c0 = theta[0] + 1j * theta[1]
    evc0 = c0 / np.linalg.norm(c0)
    c1 = theta[2] + 1j * theta[3]
    c1 = c1 - np.vdot(evc0, c1) * evc0
    evc1 = c1 / np.linalg.norm(c1)
    lam = np.log1p(np.exp(evl))
    lam = lam / np.linalg.norm(lam)
    U = np.stack([evc0.real, evc0.imag, evc1.real, evc1.imag], axis=1)
    return U, lam  # f64 (128, 4), f64 (2,)


def _make_u_planes(U: np.ndarray) -> np.ndarray:
    """Zero-shifted DoubleRow stationary planes, fp8, scaled by U_SCALE.

    Slot i covers out partitions 4i..4i+3 = rows [g1_0, g2_0, g1_1, g2_1]:
      plane 0 (applied to R): [ur0, ui0, ur1, ui1]
      plane 1 (applied to I): [ui0, -ur0, ui1, -ur1]
    """
    A = U * U_SCALE
    Bp = np.stack([U[:, 1], -U[:, 0], U[:, 3], -U[:, 2]], axis=1) * U_SCALE
    u_np = np.zeros((N, 2, MM_PER_GROUP, GROUP_MS), dtype=np.float32)
    for i in range(MM_PER_GROUP):
        u_np[:, 0, i, 4 * i:4 * i + 4] = A
        u_np[:, 1, i, 4 * i:4 * i + 4] = Bp
    return np.ascontiguousarray(
        u_np.reshape(N, 2, MM_PER_GROUP * GROUP_MS)
    ).astype(ml_dtypes.float8_e4m3)


def _pack_stream(basis_re_k: np.ndarray, basis_im_k: np.ndarray) -> dict:
    """fp8-cast + pack one core's slice into per-chunk-size xs arrays.

    Slot s covers m = 4s+j;  xs{n}[idx, a, q, r, j*N + b], r=0: R, r=1: I.
    """
    R = np.asarray(basis_re_k, dtype=np.float32).astype(ml_dtypes.float8_e4m3)
    I = np.asarray(basis_im_k, dtype=np.float32).astype(ml_dtypes.float8_e4m3)
    # [s, r, j, a, b]
    X = np.stack([R.reshape(N_SLOTS, PAIRS_PER_MM, N, N),
                  I.reshape(N_SLOTS, PAIRS_PER_MM, N, N)], axis=1)
    chunks, size_counts = _chunk_layout()
    bufs = {n: np.empty((cnt, N, n, 2, PAIRS_PER_MM * N),
                        dtype=ml_dtypes.float8_e4m3)
            for n, cnt in size_counts.items()}
    for ring, n, start, name, idx in chunks:
        blk = X[start:start + n]                    # [q, r, j, a, b]
        blk = np.transpose(blk, (3, 0, 1, 2, 4))    # [a, q, r, j, b]
        bufs[n][idx] = blk.reshape(N, n, 2, PAIRS_PER_MM * N)
    return {f"xs{n}": v for n, v in bufs.items()}


def _decode(t_raw: np.ndarray, U: np.ndarray, lam: np.ndarray) -> float:
    """Host stage 2 + combine for one core's t_out. Returns partial loss."""
    # t_raw[g, 4i + x', j*128 + b], m = g*32 + i*4 + j
    G = t_raw.reshape(N_GROUPS, MM_PER_GROUP, 4, PAIRS_PER_MM, N).astype(
        np.float64)
    G = np.transpose(G, (0, 1, 3, 2, 4)).reshape(M_LOCAL, 4, N) / U_SCALE
    u0 = U[:, 0] + 1j * U[:, 1]
    u1 = U[:, 2] + 1j * U[:, 3]
    F0 = (G[:, 0, :] + 1j * G[:, 1, :]) @ np.conj(u0)
    F1 = (G[:, 2, :] + 1j * G[:, 3, :]) @ np.conj(u1)
    v = lam[0] * F0 - lam[1] * F1
    return float(np.sum(v.real ** 2 + v.imag ** 2))


def _make_in_maps(basis_re, basis_im, theta, evl):
    U, lam = _host_prep(theta, evl)
    u_packed = _make_u_planes(U)
    in_maps = []
    for k in range(N_CORES):
        sl = slice(k * M_LOCAL, (k + 1) * M_LOCAL)
        im = _pack_stream(basis_re[sl], basis_im[sl])
        im["u"] = u_packed
        in_maps.append(im)
    return in_maps, U, lam


def _run_device(in_maps, **kwargs):
    from concourse.bass_utils import run_bass_kernel_spmd
    nc = _get_nc()
    return run_bass_kernel_spmd(nc, in_maps, list(range(N_CORES)), **kwargs)


def kernel(basis_re, basis_im, theta, evl) -> np.ndarray:
    in_maps, U, lam = _make_in_maps(basis_re, basis_im, theta, evl)
    res = _run_device(in_maps)
    total = 0.0
    for k in range(N_CORES):
        total += _decode(res.results[k]["t_out"], U, lam)
    return np.float32(total)
